# revision 2
# baseline (speedup 1.0000x reference)
"""Trainium2 Bass kernel v2 for nn_Attention_8143257993917.

Multi-head attention (packed QKV + RoPE + additive bias + softmax + head_mask
+ o_proj), B=4, S=2048, D=1024, H=16 heads, fp32 I/O.

Sharding: 8 cores = 4 batches x 2 head-groups; core c -> batch c//2, head
group c%2 (8 heads). Host sums the two per-batch partials and adds o_b.

v2 design vs baseline (673 us):
- Single Q/K projection + RoPE via DVE stream_shuffle instead of twin
  projections with host-rotated weights (saves 131k PE cycles/core).
  Head dims are host-permuted so rotate_half partners sit on adjacent
  partitions (mask[i]=i^1 within 32-partition quadrants); the rotate sign
  is folded into a host-prepared signed sin table; scores/PV are invariant
  to the shared permutation.
- Phase interleaving: projections for later head-pairs, o_proj tiles, and
  (when real work runs out) dummy matmuls are woven between the score/PV
  matmuls so the PE never idles (idle gaps reset the DVFS ramp: PE drops
  2.4 -> 1.2 GHz, which is where the baseline lost ~200us) and so scores
  are always < 50% of PE columns, keeping ScalarE exp off the critical
  path (Act exp throughput == PE score production rate when scores are
  50% of columns; it needs slack).
- Paired PSUM score tile [128, 1024] = h0|h1 512-col halves, double
  buffered by kc parity; one exp instruction per kc covers both heads.
  ScalarE runs exp ONLY; eb-multiplies on DVE (fp16 2x mode), rope
  cos/sin STTs + ctx/o evictions on GpSimd, shuffle/norm/recip on DVE.
- exp(bias) column block streamed per unit (16 KB/partition, double
  buffered, prefetched one unit ahead).
"""

import sys

sys.path.insert(0, "/opt/trn_rl_repo")

import numpy as np

_CACHE = {}

H = 16
HPC = 8  # heads per core
G = 2    # head groups


def build_nc(S=2048, D=1024):
    import concourse.bass as bass  # noqa: F401
    from concourse import bacc
    import concourse.mybir as mybir
    import concourse.tile as tile

    F32 = mybir.dt.float32
    F16 = mybir.dt.float16
    AF = mybir.ActivationFunctionType
    ALU = mybir.AluOpType

    P = 128
    DC = D // P           # 8 contraction chunks
    KC = S // P           # 16 k chunks
    NQ = 512
    QC = S // NQ          # 4 q chunks
    FPC = HPC * 64        # 512 features/core per tensor
    FT = HPC // 2         # 4 f-tiles (head pairs)

    SHUF_MASK = [i ^ 1 for i in range(32)]

    nc = bacc.Bacc("TRN2", target_bir_lowering=False, debug=False, num_devices=8)

    hT = nc.dram_tensor("hT", [D, S], F16, kind="ExternalInput")
    wqk = nc.dram_tensor("wqk", [D, 2 * FPC], F16, kind="ExternalInput")
    bqk = nc.dram_tensor("bqk", [16 * P], F32, kind="ExternalInput")
    wvT = nc.dram_tensor("wvT", [D, FPC], F16, kind="ExternalInput")
    bv = nc.dram_tensor("bv", [FPC], F16, kind="ExternalInput")
    cosr = nc.dram_tensor("cosr", [P, S], F16, kind="ExternalInput")
    sinr = nc.dram_tensor("sinr", [P, S], F16, kind="ExternalInput")
    expbT = nc.dram_tensor("expbT", [S, S], F16, kind="ExternalInput")
    owT = nc.dram_tensor("owT", [FPC, D], F16, kind="ExternalInput")
    outT = nc.dram_tensor("outT", [D, S], F32, kind="ExternalOutput")

    hT_r = hT.ap().rearrange("(o p) t -> p o t", p=P)
    wqk_r = wqk.ap().rearrange("(o p) f -> p o f", p=P)
    wv_r = wvT.ap().rearrange("(o p) f -> p o f", p=P)
    ow_r = owT.ap().rearrange("(o p) f -> p o f", p=P)
    b_r = bqk.ap().rearrange("(o p) -> p o", p=P)
    eb_r = expbT.ap().rearrange("(kc p) q -> p kc q", p=P)

    with tile.TileContext(nc) as tc:
        with (
            tc.tile_pool(name="cst", bufs=1) as cst,
            tc.tile_pool(name="big", bufs=1) as big,
            tc.tile_pool(name="ps", bufs=1, space="PSUM") as pps,
            tc.tile_pool(name="peb", bufs=2) as peb,
            tc.tile_pool(name="wk", bufs=1) as wk,
            tc.tile_pool(name="dram", bufs=4, space="DRAM") as dpool,
        ):
            ones1 = cst.tile([1, P], F16)
            nc.vector.memset(ones1[:], 1.0)
            dmv = cst.tile([1, NQ], F16)
            nc.vector.memset(dmv[:], 0.0)
            eshift = cst.tile([P, 1], F32)
            nc.vector.memset(eshift[:], -12.0)
            b_sb = cst.tile([P, 16], F32)
            nc.sync.dma_start(b_sb[:], b_r)
            bv_sb = cst.tile([1, FPC], F16)
            nc.sync.dma_start(bv_sb[:], bv.ap()[None, :])
            cos_sb = big.tile([P, S], F16)
            nc.sync.dma_start(cos_sb[:], cosr.ap())
            sin_sb = big.tile([P, S], F16)
            nc.sync.dma_start(sin_sb[:], sinr.ap())
            h_sb = big.tile([P, DC, S], F16)
            wqk_sb = big.tile([P, DC, 2 * FPC], F16)
            wv_sb = big.tile([P, DC, FPC], F16)
            for dc in range(DC):  # split big DMAs so consumers start early
                nc.sync.dma_start(wqk_sb[:, dc], wqk_r[:, dc])
                nc.sync.dma_start(h_sb[:, dc], hT_r[:, dc])
            for dc in range(DC):
                nc.sync.dma_start(wv_sb[:, dc], wv_r[:, dc])
            ow_sb = big.tile([P, FT, D], F16)
            nc.sync.dma_start(ow_sb[:], ow_r)
            qk_sb = big.tile([P, 2 * FT, S], F16)  # slots: Q ft0-3, K ft4-7
            v_sb = big.tile([P, KC, HPC, 66], F16)  # col 64 = ones (denoms)
            ctxT = big.tile([P, FT, S], F16)
            nc.vector.memset(v_sb[:, :, :, 64:65], 1.0)

            pa_alt = [0]

            def pa_tile():
                t = pps.tile([P, NQ], F32, tag=f"pa{pa_alt[0]}", name="pa")
                pa_alt[0] ^= 1
                return t

            # ---------- filler generators (yield after each PE matmul) ----
            def gen_proj(qk, ft, tq):
                """Q/K projection + rope -> qk_sb[:, qk*FT+ft, tq*NQ:...]."""
                tsl = slice(tq * NQ, (tq + 1) * NQ)
                fcol = qk * FPC + ft * P
                bcol = (qk * FT + ft) * 2
                pa = pa_tile()
                for dc in range(DC):
                    nc.tensor.matmul(pa[:], wqk_sb[:, dc, fcol:fcol + P],
                                     h_sb[:, dc, tsl],
                                     start=(dc == 0), stop=(dc == DC - 1))
                    yield
                s1 = wk.tile([P, NQ], F16, tag="s1", bufs=2)
                nc.vector.tensor_scalar_add(s1[:], pa[:], b_sb[:, bcol:bcol + 1])
                sh = wk.tile([P, NQ], F16, tag="sh", bufs=2)
                nc.vector.stream_shuffle(sh[:], s1[:], SHUF_MASK)
                tca = wk.tile([P, NQ], F16, tag="tca", bufs=2)
                nc.gpsimd.tensor_mul(tca[:], s1[:], cos_sb[:, tsl])
                tcb = wk.tile([P, NQ], F16, tag="tcb", bufs=2)
                nc.gpsimd.tensor_mul(tcb[:], sh[:], sin_sb[:, tsl])
                nc.gpsimd.tensor_add(qk_sb[:, qk * FT + ft, tsl], tca[:], tcb[:])

            def gen_vtile(tt):
                """V projection t-tile -> v_sb[:, tt] (+bias/mask, fp16)."""
                pv = pa_tile()
                for dc in range(DC):
                    nc.tensor.matmul(pv[:], h_sb[:, dc, tt * P:(tt + 1) * P],
                                     wv_sb[:, dc], start=(dc == 0), stop=False)
                    yield
                nc.tensor.matmul(pv[:], ones1[:], bv_sb[:], start=False, stop=True)
                yield
                nc.vector.tensor_copy(v_sb[:, tt, :, 0:64], pv[:])

            def gen_oproj(qc, ot):
                """o_proj tile -> outT[ot*P:(ot+1)*P, qc*NQ:...]."""
                qsl = slice(qc * NQ, (qc + 1) * NQ)
                po = pa_tile()
                for fc in range(FT):
                    nc.tensor.matmul(po[:], ow_sb[:, fc, ot * P:(ot + 1) * P],
                                     ctxT[:, fc, qsl],
                                     start=(fc == 0), stop=(fc == FT - 1))
                    yield
                o_sb = wk.tile([P, NQ], F32, tag="osb", bufs=2)
                nc.vector.tensor_copy(o_sb[:], po[:])
                nc.sync.dma_start(outT.ap()[ot * P:(ot + 1) * P, qsl], o_sb[:])

            def gen_dummy(n):
                """Keep-alive matmuls: pace the PE without real work."""
                for _ in range(n):
                    pd = pa_tile()
                    nc.tensor.matmul(pd[:], ones1[:], dmv[:], start=True, stop=True)
                    yield

            def emit_pv(ctx, hp, kp, u2q, stop):
                # kp==0 runs full-contraction (start=True initializes PSUM);
                # later kp split into 64-row halves on disjoint PE row tiles
                # so the h0/h1 halves overlap like the score pairs do.
                for hi in range(2):
                    h = 2 * hp + hi
                    nc.tensor.matmul(ctx[0:65, hi * NQ:(hi + 1) * NQ],
                                     v_sb[:, kp, h, 0:65],
                                     u2q[kp][:, hi * NQ:(hi + 1) * NQ],
                                     start=(kp == 0), stop=(stop and hi == 1))

            # ---------- static fill schedule (hp-major units) -------------
            # unit u = hp*QC + qc. Needs: B(hp,qc) <- K ft(hp) all tq, Q
            # ft(hp) tq=qc, V all (PV chases V in u0). C(qc) after u= 12+qc.
            def projs(qk, ft, tqs):
                return [gen_proj(qk, ft, tq) for tq in tqs]

            FILL = [
                [gen_vtile(tt) for tt in range(8, 16)] + projs(0, 0, [1]),   # u0
                projs(0, 0, [2]) + projs(1, 1, [0, 1]),                      # u1
                projs(0, 0, [3]) + projs(1, 1, [2, 3]),                      # u2
                projs(0, 1, [0, 1, 2, 3]),                                   # u3
                projs(1, 2, [0, 1, 2]),                                      # u4
                projs(1, 2, [3]) + projs(0, 2, [0, 1]),                      # u5
                projs(0, 2, [2, 3]) + projs(1, 3, [0]),                      # u6
                projs(1, 3, [1, 2, 3]),                                      # u7
                projs(0, 3, [0, 1, 2]),                                      # u8
                projs(0, 3, [3]) + [gen_dummy(8)],                           # u9
                [gen_dummy(12)],                                             # u10
                [gen_dummy(12)],                                             # u11
                [gen_dummy(12)],                                             # u12
                [gen_dummy(4)] + [gen_oproj(0, ot) for ot in range(6)],      # u13
                [gen_oproj(0, ot) for ot in (6, 7)] +
                [gen_oproj(1, ot) for ot in range(4)] + [gen_dummy(2)],      # u14
                [gen_oproj(1, ot) for ot in (4, 5, 6, 7)] +
                [gen_oproj(2, ot) for ot in (0, 1)] + [gen_dummy(2)],        # u15
            ]
            TRAIL = [gen_dummy(16)] + [gen_oproj(2, ot) for ot in range(2, 8)] + \
                [gen_oproj(3, ot) for ot in range(8)]

            def pump(gens, n):
                while n > 0 and gens:
                    try:
                        next(gens[0])
                        n -= 1
                    except StopIteration:
                        gens.pop(0)
                return n

            def fill_mm_count(u):
                # mm counts per unit (for even spread); keep in sync w/ FILL
                return [80, 24, 24, 32, 24, 24, 24, 24, 24, 16, 12, 12,
                        12, 28, 26, 26][u]

            # ---------- pre phase ----------------------------------------
            eb_tiles = {}

            def prefetch_eb(u):
                if u >= 16:
                    return
                qc = u % QC
                t = peb.tile([P, KC, NQ], F16, tag="eb", name=f"eb{u}")
                for kh in range(0, KC, 4):  # split across queues: low latency
                    nc.sync.dma_start(t[:, kh:kh + 4],
                                      eb_r[:, kh:kh + 4, qc * NQ:(qc + 1) * NQ])
                eb_tiles[u] = t

            prefetch_eb(0)
            prefetch_eb(1)

            # K/Q first so their Pool-side rope drains before B starts
            pre = projs(1, 0, [0, 1, 2, 3]) + projs(0, 0, [0]) + \
                [gen_vtile(tt) for tt in range(8)]
            while pre:
                pump(pre, 1 << 30)

            # ---------- B span -------------------------------------------
            pending_fin = []  # deferred finalize stages from the previous unit
            FIN_SLOTS = (1, 2, 5, 6, 7, 8)

            for hp in range(FT):
                for qc in range(QC):
                    u = hp * QC + qc
                    prefetch_eb(u + 2)
                    eb = eb_tiles.pop(u)
                    gens = FILL[u]
                    n_mm = fill_mm_count(u)
                    ft = hp
                    qsl = slice(qc * NQ, (qc + 1) * NQ)
                    ctx = None  # allocated lazily at kc==2 (first PV)
                    u2q = []  # PV runs 2 kc behind scores/exp for chain slack
                    done = 0
                    for kc in range(KC):
                        psS = pps.tile([P, 2 * NQ], F32, tag=f"s{kc % 2}",
                                       name="psS")
                        for hi in range(2):
                            base = 64 * hi
                            nc.tensor.matmul(
                                psS[:, hi * NQ:(hi + 1) * NQ],
                                qk_sb[base:base + 64, FT + ft, kc * P:(kc + 1) * P],
                                qk_sb[base:base + 64, ft, qsl],
                                start=True, stop=True)
                        u_t = wk.tile([P, 2 * NQ], F16, tag="u", bufs=2)
                        nc.scalar.activation(u_t[:], psS[:], AF.Exp,
                                             bias=eshift[:])
                        u2 = wk.tile([P, 2 * NQ], F16, tag="u2", bufs=4)
                        nc.vector.tensor_mul(
                            u2[:].rearrange("p (a b) -> p a b", a=2),
                            u_t[:].rearrange("p (a b) -> p a b", a=2),
                            eb[:, kc:kc + 1, :].broadcast_to([P, 2, NQ]))
                        hold = 10 if u >= 13 else 0  # C fills wait for norms
                        if kc >= hold:
                            want = (n_mm * (kc + 1 - hold)) // (KC - hold) - done
                            done += want - pump(gens, want)
                        if kc in FIN_SLOTS and pending_fin:
                            pending_fin.pop(0)()
                        u2q.append(u2)
                        if kc >= 3:
                            if ctx is None:
                                ctx = pps.tile([P, 2 * NQ], F32, tag="ctx",
                                               name="ctx")
                            emit_pv(ctx, hp, kc - 3, u2q, stop=False)
                    for kp in (KC - 3, KC - 2, KC - 1):
                        emit_pv(ctx, hp, kp, u2q, stop=(kp == KC - 1))
                    while gens:  # leftover fill (shouldn't trigger)
                        pump(gens, 1 << 30)

                    # finalize, deferred into the next unit's odd-kc slots as
                    # small stages so no engine queue gets a multi-us clump:
                    # evict ctx (ScalarE), denom reciprocal via DRAM
                    # round-trip broadcast (HW DMA queues), normalize (Pool).
                    st = {}

                    def f_evict(ctx=ctx):
                        st["ctx_sb"] = wk.tile([65, 2 * NQ], F32, tag="ctxe",
                                               bufs=1, name="ctx_sb")
                        nc.vector.tensor_copy(st["ctx_sb"][:], ctx[0:65, :])

                    def f_rd():
                        st["rd"] = dpool.tile([2 * NQ], F32, name="rd")
                        nc.gpsimd.dma_start(st["rd"][None, :],
                                            st["ctx_sb"][64:65, :])
                        st["rsq"] = wk.tile([32, 2 * NQ // 32], F32, tag="rsq",
                                            bufs=2, name="rsq")
                        nc.gpsimd.dma_start(
                            st["rsq"][:], st["rd"].rearrange("(a b) -> a b",
                                                             a=32))

                    def f_recip():
                        st["rrec"] = wk.tile([32, 2 * NQ // 32], F16,
                                             tag="rrec", bufs=2, name="rrec")
                        with nc.allow_low_precision(reason="1/denom fp16"):
                            nc.vector.reciprocal(st["rrec"][:], st["rsq"][:])

                    def f_rb():
                        st["rd2"] = dpool.tile([2 * NQ], F16, name="rd2")
                        nc.gpsimd.dma_start(
                            st["rd2"].rearrange("(a b) -> a b", a=32),
                            st["rrec"][:])
                        st["rb"] = wk.tile([64, 2 * NQ], F16, tag="rb", bufs=2, name="rb")
                        nc.gpsimd.dma_start(st["rb"][:],
                                            st["rd2"].partition_broadcast(64))

                    def f_norm(hi, hp=hp, qsl=qsl):
                        base = 64 * hi
                        nc.gpsimd.tensor_mul(
                            ctxT[base:base + 64, hp, qsl],
                            st["ctx_sb"][0:64, hi * NQ:(hi + 1) * NQ],
                            st["rb"][:, hi * NQ:(hi + 1) * NQ])

                    pending_fin.extend([
                        f_evict, f_rd, f_recip, f_rb,
                        lambda: f_norm(0), lambda: f_norm(1)])

            # ---------- trail: finalize last unit + last o_proj column ----
            for f in pending_fin:
                f()
            pending_fin = []
            while TRAIL:
                pump(TRAIL, 1 << 30)

    nc.compile()
    return nc


def make_core_inputs(hidden_states, attention_bias, rope_cos, rope_sin,
                     head_mask, qkv_w, qkv_b, o_w, S=2048, D=1024):
    """Host-side sharding + layout prep. Returns list of 8 input dicts."""
    f32, f16 = np.float32, np.float16
    hidden_states = np.asarray(hidden_states, f32)
    attention_bias = np.asarray(attention_bias, f32)
    rope_cos = np.asarray(rope_cos, f32)
    rope_sin = np.asarray(rope_sin, f32)
    head_mask = np.asarray(head_mask, f32).reshape(-1)
    qkv_w = np.asarray(qkv_w, f32)
    qkv_b = np.asarray(qkv_b, f32)
    o_w = np.asarray(o_w, f32)

    FPC = HPC * 64
    F = H * 64

    # d-permutation: position p = 2*(d%32) + d//32  (rotate partners adjacent)
    perm = np.empty(64, np.int64)
    for d in range(64):
        perm[2 * (d % 32) + d // 32] = d
    x1 = np.arange(128) ^ 1      # partition pair-swap (within 64-halves too)
    sgn64 = np.where(np.arange(64) % 2 == 0, -1.0, 1.0).astype(f32)

    def perm_rows(w):
        # w: [FPC(, D)] rows f = h*64 + d -> rows h*64 + p with p-order
        w = w.reshape(HPC, 64, -1)
        out = w[:, perm]
        return out.reshape(HPC * 64, -1)

    cos64 = rope_cos[0, :, 0, :]           # [S, 64]
    sin64 = rope_sin[0, :, 0, :]
    cos_p = cos64[:, perm].T               # [64, S] p-order
    sin_p = (sin64[:, perm] * sgn64[None, :]).T
    cosr = np.concatenate([cos_p, cos_p], axis=0).astype(f16)   # [128, S]
    sinr = np.concatenate([sin_p, sin_p], axis=0).astype(f16)

    in_maps = []
    for c in range(8):
        b, g = divmod(c, G)
        fs = slice(g * FPC, (g + 1) * FPC)
        wq = perm_rows(qkv_w[F * 0:F * 1][fs])
        wk_ = perm_rows(qkv_w[F * 1:F * 2][fs])
        bq = perm_rows(qkv_b[F * 0:F * 1][fs, None]).ravel()
        bk = perm_rows(qkv_b[F * 1:F * 2][fs, None]).ravel()
        wv = qkv_w[F * 2:F * 3][fs].copy()
        bvv = qkv_b[F * 2:F * 3][fs].copy()
        mask = head_mask[g * HPC:(g + 1) * HPC]
        wv *= np.repeat(mask, 64)[:, None]
        bvv *= np.repeat(mask, 64)

        wqk = np.concatenate([wq.T, wk_.T], axis=1)   # [D, 2*FPC]
        # bias scalars [16 cols x 128]: (qk*4+ft)*2 + {plain, shuffled}
        bcols = np.empty((16, 128), f32)
        for qk, bvec in ((0, bq), (1, bk)):
            for ft in range(4):
                seg = bvec[ft * 128:(ft + 1) * 128]
                bcols[(qk * 4 + ft) * 2 + 0] = seg
                bcols[(qk * 4 + ft) * 2 + 1] = seg[x1]
        bT = np.ascontiguousarray(attention_bias[b, 0].T)
        m = {
            "hT": np.ascontiguousarray(hidden_states[b].T).astype(f16),
            "wqk": np.ascontiguousarray(wqk).astype(f16),
            "bqk": np.ascontiguousarray(bcols.ravel()),
            "wvT": np.ascontiguousarray(wv.T).astype(f16),
            "bv": np.ascontiguousarray(bvv).astype(f16),
            "cosr": np.ascontiguousarray(cosr),
            "sinr": np.ascontiguousarray(sinr),
            "expbT": np.exp(bT).astype(f16),
            "owT": np.ascontiguousarray(o_w[:, g * FPC:(g + 1) * FPC].T).astype(f16),
        }
        in_maps.append(m)
    return in_maps


def kernel(hidden_states, attention_bias, rope_cos, rope_sin, head_mask,
           qkv_w, qkv_b, o_w, o_b, **_unused):
    from concourse.bass_utils import run_bass_kernel_spmd

    B, S, D = hidden_states.shape
    if "nc" not in _CACHE:
        _CACHE["nc"] = build_nc(S=S, D=D)
    nc = _CACHE["nc"]

    in_maps = make_core_inputs(hidden_states, attention_bias, rope_cos,
                               rope_sin, head_mask, qkv_w, qkv_b, o_w,
                               S=S, D=D)
    res = run_bass_kernel_spmd(nc, in_maps, list(range(8)))
    _CACHE["last_results"] = res

    o_b = np.asarray(o_b, np.float32)
    out = np.empty((B, S, D), np.float32)
    for b in range(B):
        acc = res.results[2 * b]["outT"].T + res.results[2 * b + 1]["outT"].T
        out[b] = acc + o_b[None, :]
    return out


# revision 3
# speedup vs baseline: 1.0219x; 1.0219x over previous
"""Trainium2 Bass kernel v2 for nn_Attention_8143257993917.

Multi-head attention (packed QKV + RoPE + additive bias + softmax + head_mask
+ o_proj), B=4, S=2048, D=1024, H=16 heads, fp32 I/O.

Sharding: 8 cores = 4 batches x 2 head-groups; core c -> batch c//2, head
group c%2 (8 heads). Host sums the two per-batch partials and adds o_b.

v2 design vs baseline (673 us):
- Single Q/K projection + RoPE via DVE stream_shuffle instead of twin
  projections with host-rotated weights (saves 131k PE cycles/core).
  Head dims are host-permuted so rotate_half partners sit on adjacent
  partitions (mask[i]=i^1 within 32-partition quadrants); the rotate sign
  is folded into a host-prepared signed sin table; scores/PV are invariant
  to the shared permutation.
- Phase interleaving: projections for later head-pairs, o_proj tiles, and
  (when real work runs out) dummy matmuls are woven between the score/PV
  matmuls so the PE never idles (idle gaps reset the DVFS ramp: PE drops
  2.4 -> 1.2 GHz, which is where the baseline lost ~200us) and so scores
  are always < 50% of PE columns, keeping ScalarE exp off the critical
  path (Act exp throughput == PE score production rate when scores are
  50% of columns; it needs slack).
- Paired PSUM score tile [128, 1024] = h0|h1 512-col halves, double
  buffered by kc parity; one exp instruction per kc covers both heads.
  ScalarE runs exp ONLY; eb-multiplies on DVE (fp16 2x mode), rope
  cos/sin STTs + ctx/o evictions on GpSimd, shuffle/norm/recip on DVE.
- exp(bias) column block streamed per unit (16 KB/partition, double
  buffered, prefetched one unit ahead).
"""

import sys

sys.path.insert(0, "/opt/trn_rl_repo")

import numpy as np

_CACHE = {}

H = 16
HPC = 8  # heads per core
G = 2    # head groups


def build_nc(S=2048, D=1024):
    import concourse.bass as bass  # noqa: F401
    from concourse import bacc
    import concourse.mybir as mybir
    import concourse.tile as tile

    F32 = mybir.dt.float32
    F16 = mybir.dt.float16
    AF = mybir.ActivationFunctionType
    ALU = mybir.AluOpType

    P = 128
    DC = D // P           # 8 contraction chunks
    KC = S // P           # 16 k chunks
    NQ = 512
    QC = S // NQ          # 4 q chunks
    FPC = HPC * 64        # 512 features/core per tensor
    FT = HPC // 2         # 4 f-tiles (head pairs)

    SHUF_MASK = [i ^ 1 for i in range(32)]

    nc = bacc.Bacc("TRN2", target_bir_lowering=False, debug=False, num_devices=8)

    hT = nc.dram_tensor("hT", [D, S], F16, kind="ExternalInput")
    wqk = nc.dram_tensor("wqk", [D, 2 * FPC], F16, kind="ExternalInput")
    bqk = nc.dram_tensor("bqk", [16 * P], F32, kind="ExternalInput")
    wvT = nc.dram_tensor("wvT", [D, FPC], F16, kind="ExternalInput")
    bv = nc.dram_tensor("bv", [FPC], F16, kind="ExternalInput")
    cosr = nc.dram_tensor("cosr", [P, S], F16, kind="ExternalInput")
    sinr = nc.dram_tensor("sinr", [P, S], F16, kind="ExternalInput")
    expbT = nc.dram_tensor("expbT", [S, S], F16, kind="ExternalInput")
    owT = nc.dram_tensor("owT", [FPC, D], F16, kind="ExternalInput")
    outT = nc.dram_tensor("outT", [D, S], F32, kind="ExternalOutput")

    hT_r = hT.ap().rearrange("(o p) t -> p o t", p=P)
    wqk_r = wqk.ap().rearrange("(o p) f -> p o f", p=P)
    wv_r = wvT.ap().rearrange("(o p) f -> p o f", p=P)
    ow_r = owT.ap().rearrange("(o p) f -> p o f", p=P)
    b_r = bqk.ap().rearrange("(o p) -> p o", p=P)
    eb_r = expbT.ap().rearrange("(kc p) q -> p kc q", p=P)

    with tile.TileContext(nc) as tc:
        with (
            tc.tile_pool(name="cst", bufs=1) as cst,
            tc.tile_pool(name="big", bufs=1) as big,
            tc.tile_pool(name="ps", bufs=1, space="PSUM") as pps,
            tc.tile_pool(name="peb", bufs=2) as peb,
            tc.tile_pool(name="wk", bufs=1) as wk,
            tc.tile_pool(name="dram", bufs=4, space="DRAM") as dpool,
        ):
            ones1 = cst.tile([1, P], F16)
            nc.vector.memset(ones1[:], 1.0)
            dmv = cst.tile([1, NQ], F16)
            nc.vector.memset(dmv[:], 0.0)
            eshift = cst.tile([P, 1], F32)
            nc.vector.memset(eshift[:], -12.0)
            b_sb = cst.tile([P, 16], F32)
            nc.sync.dma_start(b_sb[:], b_r)
            bv_sb = cst.tile([1, FPC], F16)
            nc.sync.dma_start(bv_sb[:], bv.ap()[None, :])
            cos_sb = big.tile([P, S], F16)
            nc.sync.dma_start(cos_sb[:], cosr.ap())
            sin_sb = big.tile([P, S], F16)
            nc.sync.dma_start(sin_sb[:], sinr.ap())
            h_sb = big.tile([P, DC, S], F16)
            wqk_sb = big.tile([P, DC, 2 * FPC], F16)
            wv_sb = big.tile([P, DC, FPC], F16)
            for dc in range(DC):  # split big DMAs so consumers start early
                nc.sync.dma_start(wqk_sb[:, dc], wqk_r[:, dc])
                nc.sync.dma_start(h_sb[:, dc], hT_r[:, dc])
            for dc in range(DC):
                nc.sync.dma_start(wv_sb[:, dc], wv_r[:, dc])
            ow_sb = big.tile([P, FT, D], F16)
            nc.sync.dma_start(ow_sb[:], ow_r)
            qk_sb = big.tile([P, 2 * FT, S], F16)  # slots: Q ft0-3, K ft4-7
            v_sb = big.tile([P, KC, HPC, 66], F16)  # col 64 = ones (denoms)
            ctxT = big.tile([P, FT, S], F16)
            nc.vector.memset(v_sb[:, :, :, 64:65], 1.0)

            pa_alt = [0]

            def pa_tile():
                t = pps.tile([P, NQ], F32, tag=f"pa{pa_alt[0]}", name="pa")
                pa_alt[0] ^= 1
                return t

            # ---------- filler generators (yield after each PE matmul) ----
            def gen_proj(qk, ft, tq):
                """Q/K projection + rope -> qk_sb[:, qk*FT+ft, tq*NQ:...]."""
                tsl = slice(tq * NQ, (tq + 1) * NQ)
                fcol = qk * FPC + ft * P
                bcol = (qk * FT + ft) * 2
                pa = pa_tile()
                for dc in range(DC):
                    nc.tensor.matmul(pa[:], wqk_sb[:, dc, fcol:fcol + P],
                                     h_sb[:, dc, tsl],
                                     start=(dc == 0), stop=(dc == DC - 1))
                    yield
                s1 = wk.tile([P, NQ], F16, tag="s1", bufs=2)
                nc.vector.tensor_scalar_add(s1[:], pa[:], b_sb[:, bcol:bcol + 1])
                sh = wk.tile([P, NQ], F16, tag="sh", bufs=2)
                nc.vector.stream_shuffle(sh[:], s1[:], SHUF_MASK)
                tca = wk.tile([P, NQ], F16, tag="tca", bufs=2)
                nc.gpsimd.tensor_mul(tca[:], s1[:], cos_sb[:, tsl])
                tcb = wk.tile([P, NQ], F16, tag="tcb", bufs=2)
                nc.gpsimd.tensor_mul(tcb[:], sh[:], sin_sb[:, tsl])
                nc.gpsimd.tensor_add(qk_sb[:, qk * FT + ft, tsl], tca[:], tcb[:])

            def gen_vtile(tt):
                """V projection t-tile -> v_sb[:, tt] (+bias/mask, fp16)."""
                pv = pa_tile()
                for dc in range(DC):
                    nc.tensor.matmul(pv[:], h_sb[:, dc, tt * P:(tt + 1) * P],
                                     wv_sb[:, dc], start=(dc == 0), stop=False)
                    yield
                nc.tensor.matmul(pv[:], ones1[:], bv_sb[:], start=False, stop=True)
                yield
                nc.vector.tensor_copy(v_sb[:, tt, :, 0:64], pv[:])

            def gen_oproj(qc, ot):
                """o_proj tile -> outT[ot*P:(ot+1)*P, qc*NQ:...]."""
                qsl = slice(qc * NQ, (qc + 1) * NQ)
                po = pa_tile()
                for fc in range(FT):
                    nc.tensor.matmul(po[:], ow_sb[:, fc, ot * P:(ot + 1) * P],
                                     ctxT[:, fc, qsl],
                                     start=(fc == 0), stop=(fc == FT - 1))
                    yield
                o_sb = wk.tile([P, NQ], F32, tag="osb", bufs=2)
                nc.vector.tensor_copy(o_sb[:], po[:])
                nc.sync.dma_start(outT.ap()[ot * P:(ot + 1) * P, qsl], o_sb[:])

            def gen_dummy(n):
                """Keep-alive matmuls: pace the PE without real work."""
                for _ in range(n):
                    pd = pa_tile()
                    nc.tensor.matmul(pd[:], ones1[:], dmv[:], start=True, stop=True)
                    yield

            def emit_pv(ctx, hp, kp, u2q, stop):
                # kp==0 runs full-contraction (start=True initializes PSUM);
                # later kp split into 64-row halves on disjoint PE row tiles
                # so the h0/h1 halves overlap like the score pairs do.
                for hi in range(2):
                    h = 2 * hp + hi
                    nc.tensor.matmul(ctx[0:65, hi * NQ:(hi + 1) * NQ],
                                     v_sb[:, kp, h, 0:65],
                                     u2q[kp][:, hi * NQ:(hi + 1) * NQ],
                                     start=(kp == 0), stop=(stop and hi == 1))

            # ---------- static fill schedule (hp-major units) -------------
            # unit u = hp*QC + qc. Needs: B(hp,qc) <- K ft(hp) all tq, Q
            # ft(hp) tq=qc, V all (PV chases V in u0). C(qc) after u= 12+qc.
            def projs(qk, ft, tqs):
                return [gen_proj(qk, ft, tq) for tq in tqs]

            FILL = [
                [gen_vtile(tt) for tt in range(8, 16)] + projs(0, 0, [1]),   # u0
                projs(0, 0, [2]) + projs(1, 1, [0, 1]),                      # u1
                projs(0, 0, [3]) + projs(1, 1, [2, 3]),                      # u2
                projs(0, 1, [0, 1, 2, 3]),                                   # u3
                projs(1, 2, [0, 1, 2]),                                      # u4
                projs(1, 2, [3]) + projs(0, 2, [0, 1]),                      # u5
                projs(0, 2, [2, 3]) + projs(1, 3, [0]),                      # u6
                projs(1, 3, [1, 2, 3]),                                      # u7
                projs(0, 3, [0, 1, 2]),                                      # u8
                projs(0, 3, [3]) + [gen_dummy(8)],                           # u9
                [gen_dummy(14)],                                             # u10
                [gen_dummy(14)],                                             # u11
                [gen_dummy(14)],                                             # u12
                [gen_dummy(4)] + [gen_oproj(0, ot) for ot in range(6)],      # u13
                [gen_oproj(0, ot) for ot in (6, 7)] +
                [gen_oproj(1, ot) for ot in range(4)] + [gen_dummy(2)],      # u14
                [gen_oproj(1, ot) for ot in (4, 5, 6, 7)] +
                [gen_oproj(2, ot) for ot in (0, 1)] + [gen_dummy(2)],        # u15
            ]
            TRAIL = [gen_dummy(16)] + [gen_oproj(2, ot) for ot in range(2, 8)] + \
                [gen_oproj(3, ot) for ot in range(8)]

            def pump(gens, n):
                while n > 0 and gens:
                    try:
                        next(gens[0])
                        n -= 1
                    except StopIteration:
                        gens.pop(0)
                return n

            def fill_mm_count(u):
                # mm counts per unit (for even spread); keep in sync w/ FILL
                return [80, 24, 24, 32, 24, 24, 24, 24, 24, 16, 14, 14,
                        14, 28, 26, 26][u]

            # ---------- pre phase ----------------------------------------
            eb_tiles = {}

            def prefetch_eb(u):
                if u >= 16:
                    return
                qc = u % QC
                t = peb.tile([P, KC, NQ], F16, tag="eb", name=f"eb{u}")
                for kh in range(0, KC, 4):  # split across queues: low latency
                    nc.sync.dma_start(t[:, kh:kh + 4],
                                      eb_r[:, kh:kh + 4, qc * NQ:(qc + 1) * NQ])
                eb_tiles[u] = t

            prefetch_eb(0)
            prefetch_eb(1)

            # K/Q first so their Pool-side rope drains before B starts
            pre = projs(1, 0, [0, 1, 2, 3]) + projs(0, 0, [0]) + \
                [gen_vtile(tt) for tt in range(8)]
            while pre:
                pump(pre, 1 << 30)

            # ---------- B span -------------------------------------------
            pending_fin = []  # deferred finalize stages from the previous unit
            FIN_SLOTS = (1, 2, 5, 6, 7, 8)

            for hp in range(FT):
                for qc in range(QC):
                    u = hp * QC + qc
                    prefetch_eb(u + 2)
                    eb = eb_tiles.pop(u)
                    gens = FILL[u]
                    n_mm = fill_mm_count(u)
                    ft = hp
                    qsl = slice(qc * NQ, (qc + 1) * NQ)
                    ctx = None  # allocated lazily at kc==2 (first PV)
                    u2q = []  # PV runs 2 kc behind scores/exp for chain slack
                    done = 0
                    for kc in range(KC):
                        psS = pps.tile([P, 2 * NQ], F32, tag=f"s{kc % 2}",
                                       name="psS")
                        for hi in range(2):
                            base = 64 * hi
                            nc.tensor.matmul(
                                psS[:, hi * NQ:(hi + 1) * NQ],
                                qk_sb[base:base + 64, FT + ft, kc * P:(kc + 1) * P],
                                qk_sb[base:base + 64, ft, qsl],
                                start=True, stop=True)
                        u_t = wk.tile([P, 2 * NQ], F16, tag="u", bufs=2)
                        nc.scalar.activation(u_t[:], psS[:], AF.Exp,
                                             bias=eshift[:])
                        u2 = wk.tile([P, 2 * NQ], F16, tag="u2", bufs=4)
                        nc.vector.tensor_mul(
                            u2[:].rearrange("p (a b) -> p a b", a=2),
                            u_t[:].rearrange("p (a b) -> p a b", a=2),
                            eb[:, kc:kc + 1, :].broadcast_to([P, 2, NQ]))
                        hold = 8 if u >= 13 else 0  # C fills wait for norms
                        if kc >= hold:
                            want = (n_mm * (kc + 1 - hold)) // (KC - hold) - done
                            done += want - pump(gens, want)
                        if kc in FIN_SLOTS and pending_fin:
                            pending_fin.pop(0)()
                        u2q.append(u2)
                        if kc >= 3:
                            if ctx is None:
                                ctx = pps.tile([P, 2 * NQ], F32, tag="ctx",
                                               name="ctx")
                            emit_pv(ctx, hp, kc - 3, u2q, stop=False)
                    for kp in (KC - 3, KC - 2, KC - 1):
                        emit_pv(ctx, hp, kp, u2q, stop=(kp == KC - 1))
                    while gens:  # leftover fill (shouldn't trigger)
                        pump(gens, 1 << 30)

                    # finalize, deferred into the next unit's odd-kc slots as
                    # small stages so no engine queue gets a multi-us clump:
                    # evict ctx (ScalarE), denom reciprocal via DRAM
                    # round-trip broadcast (HW DMA queues), normalize (Pool).
                    st = {}

                    def f_evict(ctx=ctx):
                        st["ctx_sb"] = wk.tile([65, 2 * NQ], F32, tag="ctxe",
                                               bufs=1, name="ctx_sb")
                        nc.vector.tensor_copy(st["ctx_sb"][:], ctx[0:65, :])

                    def f_rd():
                        st["rd"] = dpool.tile([2 * NQ], F32, name="rd")
                        nc.sync.dma_start(st["rd"][None, :],
                                          st["ctx_sb"][64:65, :])
                        st["rsq"] = wk.tile([32, 2 * NQ // 32], F32, tag="rsq",
                                            bufs=2, name="rsq")
                        nc.sync.dma_start(
                            st["rsq"][:], st["rd"].rearrange("(a b) -> a b",
                                                             a=32))

                    def f_recip():
                        st["rrec"] = wk.tile([32, 2 * NQ // 32], F16,
                                             tag="rrec", bufs=2, name="rrec")
                        with nc.allow_low_precision(reason="1/denom fp16"):
                            nc.vector.reciprocal(st["rrec"][:], st["rsq"][:])

                    def f_rb():
                        st["rd2"] = dpool.tile([2 * NQ], F16, name="rd2")
                        nc.sync.dma_start(
                            st["rd2"].rearrange("(a b) -> a b", a=32),
                            st["rrec"][:])
                        st["rb"] = wk.tile([64, 2 * NQ], F16, tag="rb", bufs=2, name="rb")
                        nc.sync.dma_start(st["rb"][:],
                                          st["rd2"].partition_broadcast(64))

                    def f_norm(hi, hp=hp, qsl=qsl):
                        base = 64 * hi
                        nc.gpsimd.tensor_mul(
                            ctxT[base:base + 64, hp, qsl],
                            st["ctx_sb"][0:64, hi * NQ:(hi + 1) * NQ],
                            st["rb"][:, hi * NQ:(hi + 1) * NQ])

                    pending_fin.extend([
                        f_evict, f_rd, f_recip, f_rb,
                        lambda: f_norm(0), lambda: f_norm(1)])

            # ---------- trail: finalize last unit + last o_proj column ----
            for f in pending_fin:
                f()
            pending_fin = []
            while TRAIL:
                pump(TRAIL, 1 << 30)

    nc.compile()
    return nc


def make_core_inputs(hidden_states, attention_bias, rope_cos, rope_sin,
                     head_mask, qkv_w, qkv_b, o_w, S=2048, D=1024):
    """Host-side sharding + layout prep. Returns list of 8 input dicts."""
    f32, f16 = np.float32, np.float16
    hidden_states = np.asarray(hidden_states, f32)
    attention_bias = np.asarray(attention_bias, f32)
    rope_cos = np.asarray(rope_cos, f32)
    rope_sin = np.asarray(rope_sin, f32)
    head_mask = np.asarray(head_mask, f32).reshape(-1)
    qkv_w = np.asarray(qkv_w, f32)
    qkv_b = np.asarray(qkv_b, f32)
    o_w = np.asarray(o_w, f32)

    FPC = HPC * 64
    F = H * 64

    # d-permutation: position p = 2*(d%32) + d//32  (rotate partners adjacent)
    perm = np.empty(64, np.int64)
    for d in range(64):
        perm[2 * (d % 32) + d // 32] = d
    x1 = np.arange(128) ^ 1      # partition pair-swap (within 64-halves too)
    sgn64 = np.where(np.arange(64) % 2 == 0, -1.0, 1.0).astype(f32)

    def perm_rows(w):
        # w: [FPC(, D)] rows f = h*64 + d -> rows h*64 + p with p-order
        w = w.reshape(HPC, 64, -1)
        out = w[:, perm]
        return out.reshape(HPC * 64, -1)

    cos64 = rope_cos[0, :, 0, :]           # [S, 64]
    sin64 = rope_sin[0, :, 0, :]
    cos_p = cos64[:, perm].T               # [64, S] p-order
    sin_p = (sin64[:, perm] * sgn64[None, :]).T
    cosr = np.concatenate([cos_p, cos_p], axis=0).astype(f16)   # [128, S]
    sinr = np.concatenate([sin_p, sin_p], axis=0).astype(f16)

    in_maps = []
    for c in range(8):
        b, g = divmod(c, G)
        fs = slice(g * FPC, (g + 1) * FPC)
        wq = perm_rows(qkv_w[F * 0:F * 1][fs])
        wk_ = perm_rows(qkv_w[F * 1:F * 2][fs])
        bq = perm_rows(qkv_b[F * 0:F * 1][fs, None]).ravel()
        bk = perm_rows(qkv_b[F * 1:F * 2][fs, None]).ravel()
        wv = qkv_w[F * 2:F * 3][fs].copy()
        bvv = qkv_b[F * 2:F * 3][fs].copy()
        mask = head_mask[g * HPC:(g + 1) * HPC]
        wv *= np.repeat(mask, 64)[:, None]
        bvv *= np.repeat(mask, 64)

        wqk = np.concatenate([wq.T, wk_.T], axis=1)   # [D, 2*FPC]
        # bias scalars [16 cols x 128]: (qk*4+ft)*2 + {plain, shuffled}
        bcols = np.empty((16, 128), f32)
        for qk, bvec in ((0, bq), (1, bk)):
            for ft in range(4):
                seg = bvec[ft * 128:(ft + 1) * 128]
                bcols[(qk * 4 + ft) * 2 + 0] = seg
                bcols[(qk * 4 + ft) * 2 + 1] = seg[x1]
        bT = np.ascontiguousarray(attention_bias[b, 0].T)
        m = {
            "hT": np.ascontiguousarray(hidden_states[b].T).astype(f16),
            "wqk": np.ascontiguousarray(wqk).astype(f16),
            "bqk": np.ascontiguousarray(bcols.ravel()),
            "wvT": np.ascontiguousarray(wv.T).astype(f16),
            "bv": np.ascontiguousarray(bvv).astype(f16),
            "cosr": np.ascontiguousarray(cosr),
            "sinr": np.ascontiguousarray(sinr),
            "expbT": np.exp(bT).astype(f16),
            "owT": np.ascontiguousarray(o_w[:, g * FPC:(g + 1) * FPC].T).astype(f16),
        }
        in_maps.append(m)
    return in_maps


def kernel(hidden_states, attention_bias, rope_cos, rope_sin, head_mask,
           qkv_w, qkv_b, o_w, o_b, **_unused):
    from concourse.bass_utils import run_bass_kernel_spmd

    B, S, D = hidden_states.shape
    if "nc" not in _CACHE:
        _CACHE["nc"] = build_nc(S=S, D=D)
    nc = _CACHE["nc"]

    in_maps = make_core_inputs(hidden_states, attention_bias, rope_cos,
                               rope_sin, head_mask, qkv_w, qkv_b, o_w,
                               S=S, D=D)
    res = run_bass_kernel_spmd(nc, in_maps, list(range(8)))
    _CACHE["last_results"] = res

    o_b = np.asarray(o_b, np.float32)
    out = np.empty((B, S, D), np.float32)
    for b in range(B):
        acc = res.results[2 * b]["outT"].T + res.results[2 * b + 1]["outT"].T
        out[b] = acc + o_b[None, :]
    return out


# revision 4
# speedup vs baseline: 1.0431x; 1.0208x over previous
"""Trainium2 Bass kernel v2 for nn_Attention_8143257993917.

Multi-head attention (packed QKV + RoPE + additive bias + softmax + head_mask
+ o_proj), B=4, S=2048, D=1024, H=16 heads, fp32 I/O.

Sharding: 8 cores = 4 batches x 2 head-groups; core c -> batch c//2, head
group c%2 (8 heads). Host sums the two per-batch partials and adds o_b.

v2 design vs baseline (673 us):
- Single Q/K projection + RoPE via DVE stream_shuffle instead of twin
  projections with host-rotated weights (saves 131k PE cycles/core).
  Head dims are host-permuted so rotate_half partners sit on adjacent
  partitions (mask[i]=i^1 within 32-partition quadrants); the rotate sign
  is folded into a host-prepared signed sin table; scores/PV are invariant
  to the shared permutation.
- Phase interleaving: projections for later head-pairs, o_proj tiles, and
  (when real work runs out) dummy matmuls are woven between the score/PV
  matmuls so the PE never idles (idle gaps reset the DVFS ramp: PE drops
  2.4 -> 1.2 GHz, which is where the baseline lost ~200us) and so scores
  are always < 50% of PE columns, keeping ScalarE exp off the critical
  path (Act exp throughput == PE score production rate when scores are
  50% of columns; it needs slack).
- Paired PSUM score tile [128, 1024] = h0|h1 512-col halves, double
  buffered by kc parity; one exp instruction per kc covers both heads.
  ScalarE runs exp ONLY; eb-multiplies on DVE (fp16 2x mode), rope
  cos/sin STTs + ctx/o evictions on GpSimd, shuffle/norm/recip on DVE.
- exp(bias) column block streamed per unit (16 KB/partition, double
  buffered, prefetched one unit ahead).
"""

import sys

sys.path.insert(0, "/opt/trn_rl_repo")

import numpy as np

_CACHE = {}

H = 16
HPC = 8  # heads per core
G = 2    # head groups


def build_nc(S=2048, D=1024):
    import concourse.bass as bass  # noqa: F401
    from concourse import bacc
    import concourse.mybir as mybir
    import concourse.tile as tile

    F32 = mybir.dt.float32
    F16 = mybir.dt.float16
    AF = mybir.ActivationFunctionType
    ALU = mybir.AluOpType

    P = 128
    DC = D // P           # 8 contraction chunks
    KC = S // P           # 16 k chunks
    NQ = 512
    QC = S // NQ          # 4 q chunks
    FPC = HPC * 64        # 512 features/core per tensor
    FT = HPC // 2         # 4 f-tiles (head pairs)

    SHUF_MASK = [i ^ 1 for i in range(32)]

    nc = bacc.Bacc("TRN2", target_bir_lowering=False, debug=False, num_devices=8)

    hT = nc.dram_tensor("hT", [D, S], F16, kind="ExternalInput")
    wqk = nc.dram_tensor("wqk", [D, 2 * FPC], F16, kind="ExternalInput")
    bqk = nc.dram_tensor("bqk", [16 * P], F32, kind="ExternalInput")
    wvT = nc.dram_tensor("wvT", [D, FPC], F16, kind="ExternalInput")
    bv = nc.dram_tensor("bv", [FPC], F16, kind="ExternalInput")
    cosr = nc.dram_tensor("cosr", [P, S], F16, kind="ExternalInput")
    sinr = nc.dram_tensor("sinr", [P, S], F16, kind="ExternalInput")
    expbT = nc.dram_tensor("expbT", [S, S], F16, kind="ExternalInput")
    owT = nc.dram_tensor("owT", [FPC, D], F16, kind="ExternalInput")
    outT = nc.dram_tensor("outT", [D, S], F32, kind="ExternalOutput")

    hT_r = hT.ap().rearrange("(o p) t -> p o t", p=P)
    wqk_r = wqk.ap().rearrange("(o p) f -> p o f", p=P)
    wv_r = wvT.ap().rearrange("(o p) f -> p o f", p=P)
    ow_r = owT.ap().rearrange("(o p) f -> p o f", p=P)
    b_r = bqk.ap().rearrange("(o p) -> p o", p=P)
    eb_r = expbT.ap().rearrange("(kc p) q -> p kc q", p=P)

    with tile.TileContext(nc) as tc:
        with (
            tc.tile_pool(name="cst", bufs=1) as cst,
            tc.tile_pool(name="big", bufs=1) as big,
            tc.tile_pool(name="ps", bufs=1, space="PSUM") as pps,
            tc.tile_pool(name="peb", bufs=2) as peb,
            tc.tile_pool(name="wk", bufs=1) as wk,
            tc.tile_pool(name="dram", bufs=4, space="DRAM") as dpool,
        ):
            ones1 = cst.tile([1, P], F16)
            nc.vector.memset(ones1[:], 1.0)
            dmv = cst.tile([1, NQ], F16)
            nc.vector.memset(dmv[:], 0.0)
            eshift = cst.tile([P, 1], F32)
            nc.vector.memset(eshift[:], -12.0)
            b_sb = cst.tile([P, 16], F32)
            nc.sync.dma_start(b_sb[:], b_r)
            bv_sb = cst.tile([1, FPC], F16)
            nc.sync.dma_start(bv_sb[:], bv.ap()[None, :])
            cos_sb = big.tile([P, S], F16)
            nc.sync.dma_start(cos_sb[:], cosr.ap())
            sin_sb = big.tile([P, S], F16)
            nc.sync.dma_start(sin_sb[:], sinr.ap())
            h_sb = big.tile([P, DC, S], F16)
            wqk_sb = big.tile([P, DC, 2 * FPC], F16)
            wv_sb = big.tile([P, DC, FPC], F16)
            for dc in range(DC):  # split big DMAs so consumers start early
                nc.sync.dma_start(wqk_sb[:, dc], wqk_r[:, dc])
                nc.sync.dma_start(h_sb[:, dc], hT_r[:, dc])
            for dc in range(DC):
                nc.sync.dma_start(wv_sb[:, dc], wv_r[:, dc])
            ow_sb = big.tile([P, FT, D], F16)
            nc.sync.dma_start(ow_sb[:], ow_r)
            qk_sb = big.tile([P, 2 * FT, S], F16)  # slots: Q ft0-3, K ft4-7
            v_sb = big.tile([P, KC, HPC, 66], F16)  # col 64 = ones (denoms)
            ctxT = big.tile([P, FT, S], F16)
            nc.vector.memset(v_sb[:, :, :, 64:65], 1.0)

            pa_alt = [0]

            def pa_tile():
                t = pps.tile([P, NQ], F32, tag=f"pa{pa_alt[0]}", name="pa")
                pa_alt[0] ^= 1
                return t

            # ---------- filler generators (yield after each PE matmul) ----
            def gen_proj(qk, ft, tq):
                """Q/K projection + rope -> qk_sb[:, qk*FT+ft, tq*NQ:...]."""
                tsl = slice(tq * NQ, (tq + 1) * NQ)
                fcol = qk * FPC + ft * P
                bcol = (qk * FT + ft) * 2
                pa = pa_tile()
                for dc in range(DC):
                    nc.tensor.matmul(pa[:], wqk_sb[:, dc, fcol:fcol + P],
                                     h_sb[:, dc, tsl],
                                     start=(dc == 0), stop=(dc == DC - 1))
                    yield
                s1 = wk.tile([P, NQ], F16, tag="s1", bufs=2)
                nc.vector.tensor_scalar_add(s1[:], pa[:], b_sb[:, bcol:bcol + 1])
                sh = wk.tile([P, NQ], F16, tag="sh", bufs=2)
                nc.vector.stream_shuffle(sh[:], s1[:], SHUF_MASK)
                tca = wk.tile([P, NQ], F16, tag="tca", bufs=2)
                nc.gpsimd.tensor_mul(tca[:], s1[:], cos_sb[:, tsl])
                tcb = wk.tile([P, NQ], F16, tag="tcb", bufs=2)
                nc.gpsimd.tensor_mul(tcb[:], sh[:], sin_sb[:, tsl])
                nc.gpsimd.tensor_add(qk_sb[:, qk * FT + ft, tsl], tca[:], tcb[:])

            def gen_vtile(tt):
                """V projection t-tile -> v_sb[:, tt] (+bias/mask, fp16)."""
                pv = pa_tile()
                for dc in range(DC):
                    nc.tensor.matmul(pv[:], h_sb[:, dc, tt * P:(tt + 1) * P],
                                     wv_sb[:, dc], start=(dc == 0), stop=False)
                    yield
                nc.tensor.matmul(pv[:], ones1[:], bv_sb[:], start=False, stop=True)
                yield
                nc.vector.tensor_copy(v_sb[:, tt, :, 0:64], pv[:])

            def gen_oproj(qc, ot):
                """o_proj tile -> outT[ot*P:(ot+1)*P, qc*NQ:...]."""
                qsl = slice(qc * NQ, (qc + 1) * NQ)
                po = pa_tile()
                for fc in range(FT):
                    nc.tensor.matmul(po[:], ow_sb[:, fc, ot * P:(ot + 1) * P],
                                     ctxT[:, fc, qsl],
                                     start=(fc == 0), stop=(fc == FT - 1))
                    yield
                o_sb = wk.tile([P, NQ], F32, tag="osb", bufs=2)
                nc.vector.tensor_copy(o_sb[:], po[:])
                nc.sync.dma_start(outT.ap()[ot * P:(ot + 1) * P, qsl], o_sb[:])

            def gen_dummy(n):
                """Keep-alive matmuls: pace the PE without real work."""
                for _ in range(n):
                    pd = pa_tile()
                    nc.tensor.matmul(pd[:], ones1[:], dmv[:], start=True, stop=True)
                    yield

            def emit_pv(ctx, hp, kp, u2q, stop):
                # kp==0 runs full-contraction (start=True initializes PSUM);
                # later kp split into 64-row halves on disjoint PE row tiles
                # so the h0/h1 halves overlap like the score pairs do.
                for hi in range(2):
                    h = 2 * hp + hi
                    nc.tensor.matmul(ctx[0:65, hi * NQ:(hi + 1) * NQ],
                                     v_sb[:, kp, h, 0:65],
                                     u2q[kp][:, hi * NQ:(hi + 1) * NQ],
                                     start=(kp == 0), stop=(stop and hi == 1))

            # ---------- static fill schedule (hp-major units) -------------
            # unit u = hp*QC + qc. Needs: B(hp,qc) <- K ft(hp) all tq, Q
            # ft(hp) tq=qc, V all (PV chases V in u0). C(qc) after u= 12+qc.
            def projs(qk, ft, tqs):
                return [gen_proj(qk, ft, tq) for tq in tqs]

            FILL = [
                [gen_vtile(tt) for tt in range(8, 16)] + projs(0, 0, [1]),   # u0
                projs(0, 0, [2]) + projs(1, 1, [0, 1]) + [gen_dummy(2)],     # u1
                projs(0, 0, [3]) + projs(1, 1, [2, 3]) + [gen_dummy(2)],     # u2
                projs(0, 1, [0, 1, 2, 3]),                                   # u3
                projs(1, 2, [0, 1, 2]) + [gen_dummy(2)],                     # u4
                projs(1, 2, [3]) + projs(0, 2, [0, 1]) + [gen_dummy(2)],     # u5
                projs(0, 2, [2, 3]) + projs(1, 3, [0]) + [gen_dummy(2)],     # u6
                projs(1, 3, [1, 2, 3]) + [gen_dummy(2)],                     # u7
                projs(0, 3, [0, 1, 2]) + [gen_dummy(2)],                     # u8
                projs(0, 3, [3]) + [gen_dummy(8)],                           # u9
                [gen_dummy(14)],                                             # u10
                [gen_dummy(14)],                                             # u11
                [gen_dummy(14)],                                             # u12
                [gen_dummy(8)] + [gen_oproj(0, ot) for ot in range(6)],      # u13
                [gen_dummy(8)] + [gen_oproj(0, ot) for ot in (6, 7)] +
                [gen_oproj(1, ot) for ot in range(4)],                       # u14
                [gen_dummy(8)] + [gen_oproj(1, ot) for ot in (4, 5, 6, 7)] +
                [gen_oproj(2, ot) for ot in (0, 1)],                         # u15
            ]
            TRAIL = [gen_dummy(16)] + [gen_oproj(2, ot) for ot in range(2, 8)] + \
                [gen_oproj(3, ot) for ot in range(8)]

            def pump(gens, n):
                while n > 0 and gens:
                    try:
                        next(gens[0])
                        n -= 1
                    except StopIteration:
                        gens.pop(0)
                return n

            def fill_mm_count(u):
                # mm counts per unit (for even spread); keep in sync w/ FILL
                return [80, 26, 26, 32, 26, 26, 26, 26, 26, 16, 14, 14,
                        14, 32, 32, 32][u]

            # ---------- pre phase ----------------------------------------
            eb_tiles = {}

            def prefetch_eb(u):
                if u >= 16:
                    return
                qc = u % QC
                t = peb.tile([P, KC, NQ], F16, tag="eb", name=f"eb{u}")
                for kh in range(0, KC, 4):  # split across queues: low latency
                    nc.sync.dma_start(t[:, kh:kh + 4],
                                      eb_r[:, kh:kh + 4, qc * NQ:(qc + 1) * NQ])
                eb_tiles[u] = t

            prefetch_eb(0)
            prefetch_eb(1)

            # K/Q first so their Pool-side rope drains before B starts
            pre = projs(1, 0, [0, 1, 2, 3]) + projs(0, 0, [0]) + \
                [gen_vtile(tt) for tt in range(8)]
            while pre:
                pump(pre, 1 << 30)

            # ---------- B span -------------------------------------------
            pending_fin = []  # deferred finalize stages from the previous unit
            FIN_SLOTS = (1, 2, 5, 6, 7, 8)

            for hp in range(FT):
                for qc in range(QC):
                    u = hp * QC + qc
                    prefetch_eb(u + 2)
                    eb = eb_tiles.pop(u)
                    gens = FILL[u]
                    n_mm = fill_mm_count(u)
                    ft = hp
                    qsl = slice(qc * NQ, (qc + 1) * NQ)
                    ctx = None  # allocated lazily at kc==2 (first PV)
                    u2q = []  # PV runs 2 kc behind scores/exp for chain slack
                    done = 0
                    for kc in range(KC):
                        psS = pps.tile([P, 2 * NQ], F32, tag=f"s{kc % 2}",
                                       name="psS")
                        for hi in range(2):
                            base = 64 * hi
                            nc.tensor.matmul(
                                psS[:, hi * NQ:(hi + 1) * NQ],
                                qk_sb[base:base + 64, FT + ft, kc * P:(kc + 1) * P],
                                qk_sb[base:base + 64, ft, qsl],
                                start=True, stop=True)
                        u_t = wk.tile([P, 2 * NQ], F16, tag="u", bufs=2)
                        nc.scalar.activation(u_t[:], psS[:], AF.Exp,
                                             bias=eshift[:])
                        u2 = wk.tile([P, 2 * NQ], F16, tag="u2", bufs=4)
                        nc.vector.tensor_mul(
                            u2[:].rearrange("p (a b) -> p a b", a=2),
                            u_t[:].rearrange("p (a b) -> p a b", a=2),
                            eb[:, kc:kc + 1, :].broadcast_to([P, 2, NQ]))
                        if u >= 13:
                            # dummies pace iters 0-7 (Act needs period >=
                            # ~1.15us); o_proj fills wait for norms at kc8
                            if kc < 8:
                                cum = kc + 1
                            else:
                                cum = 8 + ((n_mm - 8) * (kc - 7)) // 8
                        else:
                            cum = (n_mm * (kc + 1)) // KC
                        want = cum - done
                        done += want - pump(gens, want)
                        if kc in FIN_SLOTS and pending_fin:
                            pending_fin.pop(0)()
                        u2q.append(u2)
                        if kc >= 3:
                            if ctx is None:
                                ctx = pps.tile([P, 2 * NQ], F32, tag="ctx",
                                               name="ctx")
                            emit_pv(ctx, hp, kc - 3, u2q, stop=False)
                    for kp in (KC - 3, KC - 2, KC - 1):
                        emit_pv(ctx, hp, kp, u2q, stop=(kp == KC - 1))
                    while gens:  # leftover fill (shouldn't trigger)
                        pump(gens, 1 << 30)

                    # finalize, deferred into the next unit's odd-kc slots as
                    # small stages so no engine queue gets a multi-us clump:
                    # evict ctx (ScalarE), denom reciprocal via DRAM
                    # round-trip broadcast (HW DMA queues), normalize (Pool).
                    st = {}

                    def f_evict(ctx=ctx):
                        st["ctx_sb"] = wk.tile([65, 2 * NQ], F32, tag="ctxe",
                                               bufs=1, name="ctx_sb")
                        nc.vector.tensor_copy(st["ctx_sb"][:], ctx[0:65, :])

                    def f_rd():
                        st["rd"] = dpool.tile([2 * NQ], F32, name="rd")
                        nc.sync.dma_start(st["rd"][None, :],
                                          st["ctx_sb"][64:65, :])
                        st["rsq"] = wk.tile([32, 2 * NQ // 32], F32, tag="rsq",
                                            bufs=2, name="rsq")
                        nc.sync.dma_start(
                            st["rsq"][:], st["rd"].rearrange("(a b) -> a b",
                                                             a=32))

                    def f_recip():
                        st["rrec"] = wk.tile([32, 2 * NQ // 32], F16,
                                             tag="rrec", bufs=2, name="rrec")
                        with nc.allow_low_precision(reason="1/denom fp16"):
                            nc.vector.reciprocal(st["rrec"][:], st["rsq"][:])

                    def f_rb():
                        st["rd2"] = dpool.tile([2 * NQ], F16, name="rd2")
                        nc.sync.dma_start(
                            st["rd2"].rearrange("(a b) -> a b", a=32),
                            st["rrec"][:])
                        st["rb"] = wk.tile([64, 2 * NQ], F16, tag="rb", bufs=2, name="rb")
                        nc.sync.dma_start(st["rb"][:],
                                          st["rd2"].partition_broadcast(64))

                    def f_norm(hi, hp=hp, qsl=qsl):
                        base = 64 * hi
                        nc.gpsimd.tensor_mul(
                            ctxT[base:base + 64, hp, qsl],
                            st["ctx_sb"][0:64, hi * NQ:(hi + 1) * NQ],
                            st["rb"][:, hi * NQ:(hi + 1) * NQ])

                    pending_fin.extend([
                        f_evict, f_rd, f_recip, f_rb,
                        lambda: f_norm(0), lambda: f_norm(1)])

            # ---------- trail: finalize last unit + last o_proj column ----
            for f in pending_fin:
                f()
            pending_fin = []
            while TRAIL:
                pump(TRAIL, 1 << 30)

    nc.compile()
    return nc


def make_core_inputs(hidden_states, attention_bias, rope_cos, rope_sin,
                     head_mask, qkv_w, qkv_b, o_w, S=2048, D=1024):
    """Host-side sharding + layout prep. Returns list of 8 input dicts."""
    f32, f16 = np.float32, np.float16
    hidden_states = np.asarray(hidden_states, f32)
    attention_bias = np.asarray(attention_bias, f32)
    rope_cos = np.asarray(rope_cos, f32)
    rope_sin = np.asarray(rope_sin, f32)
    head_mask = np.asarray(head_mask, f32).reshape(-1)
    qkv_w = np.asarray(qkv_w, f32)
    qkv_b = np.asarray(qkv_b, f32)
    o_w = np.asarray(o_w, f32)

    FPC = HPC * 64
    F = H * 64

    # d-permutation: position p = 2*(d%32) + d//32  (rotate partners adjacent)
    perm = np.empty(64, np.int64)
    for d in range(64):
        perm[2 * (d % 32) + d // 32] = d
    x1 = np.arange(128) ^ 1      # partition pair-swap (within 64-halves too)
    sgn64 = np.where(np.arange(64) % 2 == 0, -1.0, 1.0).astype(f32)

    def perm_rows(w):
        # w: [FPC(, D)] rows f = h*64 + d -> rows h*64 + p with p-order
        w = w.reshape(HPC, 64, -1)
        out = w[:, perm]
        return out.reshape(HPC * 64, -1)

    cos64 = rope_cos[0, :, 0, :]           # [S, 64]
    sin64 = rope_sin[0, :, 0, :]
    cos_p = cos64[:, perm].T               # [64, S] p-order
    sin_p = (sin64[:, perm] * sgn64[None, :]).T
    cosr = np.concatenate([cos_p, cos_p], axis=0).astype(f16)   # [128, S]
    sinr = np.concatenate([sin_p, sin_p], axis=0).astype(f16)

    in_maps = []
    for c in range(8):
        b, g = divmod(c, G)
        fs = slice(g * FPC, (g + 1) * FPC)
        wq = perm_rows(qkv_w[F * 0:F * 1][fs])
        wk_ = perm_rows(qkv_w[F * 1:F * 2][fs])
        bq = perm_rows(qkv_b[F * 0:F * 1][fs, None]).ravel()
        bk = perm_rows(qkv_b[F * 1:F * 2][fs, None]).ravel()
        wv = qkv_w[F * 2:F * 3][fs].copy()
        bvv = qkv_b[F * 2:F * 3][fs].copy()
        mask = head_mask[g * HPC:(g + 1) * HPC]
        wv *= np.repeat(mask, 64)[:, None]
        bvv *= np.repeat(mask, 64)

        wqk = np.concatenate([wq.T, wk_.T], axis=1)   # [D, 2*FPC]
        # bias scalars [16 cols x 128]: (qk*4+ft)*2 + {plain, shuffled}
        bcols = np.empty((16, 128), f32)
        for qk, bvec in ((0, bq), (1, bk)):
            for ft in range(4):
                seg = bvec[ft * 128:(ft + 1) * 128]
                bcols[(qk * 4 + ft) * 2 + 0] = seg
                bcols[(qk * 4 + ft) * 2 + 1] = seg[x1]
        bT = np.ascontiguousarray(attention_bias[b, 0].T)
        m = {
            "hT": np.ascontiguousarray(hidden_states[b].T).astype(f16),
            "wqk": np.ascontiguousarray(wqk).astype(f16),
            "bqk": np.ascontiguousarray(bcols.ravel()),
            "wvT": np.ascontiguousarray(wv.T).astype(f16),
            "bv": np.ascontiguousarray(bvv).astype(f16),
            "cosr": np.ascontiguousarray(cosr),
            "sinr": np.ascontiguousarray(sinr),
            "expbT": np.exp(bT).astype(f16),
            "owT": np.ascontiguousarray(o_w[:, g * FPC:(g + 1) * FPC].T).astype(f16),
        }
        in_maps.append(m)
    return in_maps


def kernel(hidden_states, attention_bias, rope_cos, rope_sin, head_mask,
           qkv_w, qkv_b, o_w, o_b, **_unused):
    from concourse.bass_utils import run_bass_kernel_spmd

    B, S, D = hidden_states.shape
    if "nc" not in _CACHE:
        _CACHE["nc"] = build_nc(S=S, D=D)
    nc = _CACHE["nc"]

    in_maps = make_core_inputs(hidden_states, attention_bias, rope_cos,
                               rope_sin, head_mask, qkv_w, qkv_b, o_w,
                               S=S, D=D)
    res = run_bass_kernel_spmd(nc, in_maps, list(range(8)))
    _CACHE["last_results"] = res

    o_b = np.asarray(o_b, np.float32)
    out = np.empty((B, S, D), np.float32)
    for b in range(B):
        acc = res.results[2 * b]["outT"].T + res.results[2 * b + 1]["outT"].T
        out[b] = acc + o_b[None, :]
    return out


# revision 5
# speedup vs baseline: 1.0443x; 1.0011x over previous
"""Trainium2 Bass kernel v2 for nn_Attention_8143257993917.

Multi-head attention (packed QKV + RoPE + additive bias + softmax + head_mask
+ o_proj), B=4, S=2048, D=1024, H=16 heads, fp32 I/O.

Sharding: 8 cores = 4 batches x 2 head-groups; core c -> batch c//2, head
group c%2 (8 heads). Host sums the two per-batch partials and adds o_b.

v2 design vs baseline (673 us):
- Single Q/K projection + RoPE via DVE stream_shuffle instead of twin
  projections with host-rotated weights (saves 131k PE cycles/core).
  Head dims are host-permuted so rotate_half partners sit on adjacent
  partitions (mask[i]=i^1 within 32-partition quadrants); the rotate sign
  is folded into a host-prepared signed sin table; scores/PV are invariant
  to the shared permutation.
- Phase interleaving: projections for later head-pairs, o_proj tiles, and
  (when real work runs out) dummy matmuls are woven between the score/PV
  matmuls so the PE never idles (idle gaps reset the DVFS ramp: PE drops
  2.4 -> 1.2 GHz, which is where the baseline lost ~200us) and so scores
  are always < 50% of PE columns, keeping ScalarE exp off the critical
  path (Act exp throughput == PE score production rate when scores are
  50% of columns; it needs slack).
- Paired PSUM score tile [128, 1024] = h0|h1 512-col halves, double
  buffered by kc parity; one exp instruction per kc covers both heads.
  ScalarE runs exp ONLY; eb-multiplies on DVE (fp16 2x mode), rope
  cos/sin STTs + ctx/o evictions on GpSimd, shuffle/norm/recip on DVE.
- exp(bias) column block streamed per unit (16 KB/partition, double
  buffered, prefetched one unit ahead).
"""

import sys

sys.path.insert(0, "/opt/trn_rl_repo")

import numpy as np

_CACHE = {}

H = 16
HPC = 8  # heads per core
G = 2    # head groups


def build_nc(S=2048, D=1024):
    import concourse.bass as bass  # noqa: F401
    from concourse import bacc
    import concourse.mybir as mybir
    import concourse.tile as tile

    F32 = mybir.dt.float32
    F16 = mybir.dt.float16
    AF = mybir.ActivationFunctionType
    ALU = mybir.AluOpType

    P = 128
    DC = D // P           # 8 contraction chunks
    KC = S // P           # 16 k chunks
    NQ = 512
    QC = S // NQ          # 4 q chunks
    FPC = HPC * 64        # 512 features/core per tensor
    FT = HPC // 2         # 4 f-tiles (head pairs)

    SHUF_MASK = [i ^ 1 for i in range(32)]

    nc = bacc.Bacc("TRN2", target_bir_lowering=False, debug=False, num_devices=8)

    hT = nc.dram_tensor("hT", [D, S], F16, kind="ExternalInput")
    wqk = nc.dram_tensor("wqk", [D, 2 * FPC], F16, kind="ExternalInput")
    bqk = nc.dram_tensor("bqk", [16 * P], F32, kind="ExternalInput")
    wvT = nc.dram_tensor("wvT", [D, FPC], F16, kind="ExternalInput")
    bv = nc.dram_tensor("bv", [FPC], F16, kind="ExternalInput")
    cosr = nc.dram_tensor("cosr", [P, S], F16, kind="ExternalInput")
    sinr = nc.dram_tensor("sinr", [P, S], F16, kind="ExternalInput")
    expbT = nc.dram_tensor("expbT", [S, S], F16, kind="ExternalInput")
    owT = nc.dram_tensor("owT", [FPC, D], F16, kind="ExternalInput")
    outT = nc.dram_tensor("outT", [D, S], F32, kind="ExternalOutput")

    hT_r = hT.ap().rearrange("(o p) t -> p o t", p=P)
    wqk_r = wqk.ap().rearrange("(o p) f -> p o f", p=P)
    wv_r = wvT.ap().rearrange("(o p) f -> p o f", p=P)
    ow_r = owT.ap().rearrange("(o p) f -> p o f", p=P)
    b_r = bqk.ap().rearrange("(o p) -> p o", p=P)
    eb_r = expbT.ap().rearrange("(kc p) q -> p kc q", p=P)

    with tile.TileContext(nc) as tc:
        with (
            tc.tile_pool(name="cst", bufs=1) as cst,
            tc.tile_pool(name="big", bufs=1) as big,
            tc.tile_pool(name="ps", bufs=1, space="PSUM") as pps,
            tc.tile_pool(name="peb", bufs=2) as peb,
            tc.tile_pool(name="wk", bufs=1) as wk,
            tc.tile_pool(name="dram", bufs=4, space="DRAM") as dpool,
        ):
            ones1 = cst.tile([1, P], F16)
            nc.vector.memset(ones1[:], 1.0)
            dmv = cst.tile([1, NQ], F16)
            nc.vector.memset(dmv[:], 0.0)
            eshift = cst.tile([P, 1], F32)
            nc.vector.memset(eshift[:], -12.0)
            b_sb = cst.tile([P, 16], F32)
            nc.sync.dma_start(b_sb[:], b_r)
            bv_sb = cst.tile([1, FPC], F16)
            nc.sync.dma_start(bv_sb[:], bv.ap()[None, :])
            cos_sb = big.tile([P, S], F16)
            nc.sync.dma_start(cos_sb[:], cosr.ap())
            sin_sb = big.tile([P, S], F16)
            nc.sync.dma_start(sin_sb[:], sinr.ap())
            h_sb = big.tile([P, DC, S], F16)
            wqk_sb = big.tile([P, DC, 2 * FPC], F16)
            wv_sb = big.tile([P, DC, FPC], F16)
            for dc in range(DC):  # split big DMAs so consumers start early
                nc.sync.dma_start(wqk_sb[:, dc], wqk_r[:, dc])
                nc.sync.dma_start(h_sb[:, dc], hT_r[:, dc])
            for dc in range(DC):
                nc.sync.dma_start(wv_sb[:, dc], wv_r[:, dc])
            ow_sb = big.tile([P, FT, D], F16)
            nc.sync.dma_start(ow_sb[:], ow_r)
            qk_sb = big.tile([P, 2 * FT, S], F16)  # slots: Q ft0-3, K ft4-7
            v_sb = big.tile([P, KC, HPC, 66], F16)  # col 64 = ones (denoms)
            ctxT = big.tile([P, FT, S], F16)
            nc.vector.memset(v_sb[:, :, :, 64:65], 1.0)

            pa_alt = [0]

            def pa_tile():
                t = pps.tile([P, NQ], F32, tag=f"pa{pa_alt[0]}", name="pa")
                pa_alt[0] ^= 1
                return t

            # ---------- filler generators (yield after each PE matmul) ----
            def gen_proj(qk, ft, tq):
                """Q/K projection + rope -> qk_sb[:, qk*FT+ft, tq*NQ:...]."""
                tsl = slice(tq * NQ, (tq + 1) * NQ)
                fcol = qk * FPC + ft * P
                bcol = (qk * FT + ft) * 2
                pa = pa_tile()
                for dc in range(DC):
                    nc.tensor.matmul(pa[:], wqk_sb[:, dc, fcol:fcol + P],
                                     h_sb[:, dc, tsl],
                                     start=(dc == 0), stop=(dc == DC - 1))
                    yield
                s1 = wk.tile([P, NQ], F16, tag="s1", bufs=2)
                nc.vector.tensor_scalar_add(s1[:], pa[:], b_sb[:, bcol:bcol + 1])
                sh = wk.tile([P, NQ], F16, tag="sh", bufs=2)
                nc.vector.stream_shuffle(sh[:], s1[:], SHUF_MASK)
                tca = wk.tile([P, NQ], F16, tag="tca", bufs=2)
                nc.gpsimd.tensor_mul(tca[:], s1[:], cos_sb[:, tsl])
                tcb = wk.tile([P, NQ], F16, tag="tcb", bufs=2)
                nc.gpsimd.tensor_mul(tcb[:], sh[:], sin_sb[:, tsl])
                nc.gpsimd.tensor_add(qk_sb[:, qk * FT + ft, tsl], tca[:], tcb[:])

            def gen_vtile(tt):
                """V projection t-tile -> v_sb[:, tt] (+bias/mask, fp16)."""
                pv = pa_tile()
                for dc in range(DC):
                    nc.tensor.matmul(pv[:], h_sb[:, dc, tt * P:(tt + 1) * P],
                                     wv_sb[:, dc], start=(dc == 0), stop=False)
                    yield
                nc.tensor.matmul(pv[:], ones1[:], bv_sb[:], start=False, stop=True)
                yield
                nc.vector.tensor_copy(v_sb[:, tt, :, 0:64], pv[:])

            trail_rot = [0]

            def trail_po():
                # trail only: psS/ctx banks are free, rotate po over 4 tags
                # so the DVE evict latency pipelines instead of serializing
                i = trail_rot[0] % 4
                trail_rot[0] += 1
                if i < 2:
                    return pps.tile([P, NQ], F32, tag=f"pa{i}", name="po")
                t = pps.tile([P, 2 * NQ], F32, tag=f"s{i - 2}", name="po")
                return t[:, 0:NQ]

            def gen_oproj(qc, ot, trail=False):
                """o_proj tile -> outT[ot*P:(ot+1)*P, qc*NQ:...]."""
                qsl = slice(qc * NQ, (qc + 1) * NQ)
                po = trail_po() if trail else pa_tile()
                for fc in range(FT):
                    nc.tensor.matmul(po[:], ow_sb[:, fc, ot * P:(ot + 1) * P],
                                     ctxT[:, fc, qsl],
                                     start=(fc == 0), stop=(fc == FT - 1))
                    yield
                o_sb = wk.tile([P, NQ], F32, tag="osb", bufs=2)
                nc.vector.tensor_copy(o_sb[:], po[:])
                nc.sync.dma_start(outT.ap()[ot * P:(ot + 1) * P, qsl], o_sb[:])

            def gen_dummy(n):
                """Keep-alive matmuls: pace the PE without real work."""
                for _ in range(n):
                    pd = pa_tile()
                    nc.tensor.matmul(pd[:], ones1[:], dmv[:], start=True, stop=True)
                    yield

            def emit_pv(ctx, hp, kp, u2q, stop):
                # kp==0 runs full-contraction (start=True initializes PSUM);
                # later kp split into 64-row halves on disjoint PE row tiles
                # so the h0/h1 halves overlap like the score pairs do.
                for hi in range(2):
                    h = 2 * hp + hi
                    nc.tensor.matmul(ctx[0:65, hi * NQ:(hi + 1) * NQ],
                                     v_sb[:, kp, h, 0:65],
                                     u2q[kp][:, hi * NQ:(hi + 1) * NQ],
                                     start=(kp == 0), stop=(stop and hi == 1))

            # ---------- static fill schedule (hp-major units) -------------
            # unit u = hp*QC + qc. Needs: B(hp,qc) <- K ft(hp) all tq, Q
            # ft(hp) tq=qc, V all (PV chases V in u0). C(qc) after u= 12+qc.
            def projs(qk, ft, tqs):
                return [gen_proj(qk, ft, tq) for tq in tqs]

            FILL = [
                [gen_vtile(tt) for tt in range(8, 16)] + projs(0, 0, [1]),   # u0
                projs(0, 0, [2]) + projs(1, 1, [0, 1]) + [gen_dummy(2)],     # u1
                projs(0, 0, [3]) + projs(1, 1, [2, 3]) + [gen_dummy(2)],     # u2
                projs(0, 1, [0, 1, 2, 3]),                                   # u3
                projs(1, 2, [0, 1, 2]) + [gen_dummy(2)],                     # u4
                projs(1, 2, [3]) + projs(0, 2, [0, 1]) + [gen_dummy(2)],     # u5
                projs(0, 2, [2, 3]) + projs(1, 3, [0]) + [gen_dummy(2)],     # u6
                projs(1, 3, [1, 2, 3]) + [gen_dummy(2)],                     # u7
                projs(0, 3, [0, 1, 2]) + [gen_dummy(2)],                     # u8
                projs(0, 3, [3]) + [gen_dummy(8)],                           # u9
                [gen_dummy(14)],                                             # u10
                [gen_dummy(14)],                                             # u11
                [gen_dummy(14)],                                             # u12
                [gen_dummy(8)] + [gen_oproj(0, ot) for ot in range(6)],      # u13
                [gen_dummy(8)] + [gen_oproj(0, ot) for ot in (6, 7)] +
                [gen_oproj(1, ot) for ot in range(4)],                       # u14
                [gen_dummy(8)] + [gen_oproj(1, ot) for ot in (4, 5, 6, 7)] +
                [gen_oproj(2, ot) for ot in (0, 1)],                         # u15
            ]
            TRAIL = [gen_dummy(16)] + [gen_oproj(2, ot) for ot in range(2, 8)] + \
                [gen_oproj(3, ot) for ot in range(8)]

            def pump(gens, n):
                while n > 0 and gens:
                    try:
                        next(gens[0])
                        n -= 1
                    except StopIteration:
                        gens.pop(0)
                return n

            def fill_mm_count(u):
                # mm counts per unit (for even spread); keep in sync w/ FILL
                return [80, 26, 26, 32, 26, 26, 26, 26, 26, 16, 14, 14,
                        14, 32, 32, 32][u]

            # ---------- pre phase ----------------------------------------
            eb_tiles = {}

            def prefetch_eb(u):
                if u >= 16:
                    return
                qc = u % QC
                t = peb.tile([P, KC, NQ], F16, tag="eb", name=f"eb{u}")
                for kh in range(0, KC, 4):  # split across queues: low latency
                    nc.sync.dma_start(t[:, kh:kh + 4],
                                      eb_r[:, kh:kh + 4, qc * NQ:(qc + 1) * NQ])
                eb_tiles[u] = t

            prefetch_eb(0)
            prefetch_eb(1)

            # K/Q first so their Pool-side rope drains before B starts
            pre = projs(1, 0, [0, 1, 2, 3]) + projs(0, 0, [0]) + \
                [gen_vtile(tt) for tt in range(8)]
            while pre:
                pump(pre, 1 << 30)

            # ---------- B span -------------------------------------------
            pending_fin = []  # deferred finalize stages from the previous unit
            FIN_SLOTS = (1, 2, 5, 6, 7, 8)

            for hp in range(FT):
                for qc in range(QC):
                    u = hp * QC + qc
                    prefetch_eb(u + 2)
                    eb = eb_tiles.pop(u)
                    gens = FILL[u]
                    n_mm = fill_mm_count(u)
                    ft = hp
                    qsl = slice(qc * NQ, (qc + 1) * NQ)
                    ctx = None  # allocated lazily at kc==2 (first PV)
                    u2q = []  # PV runs 2 kc behind scores/exp for chain slack
                    done = 0
                    for kc in range(KC):
                        psS = pps.tile([P, 2 * NQ], F32, tag=f"s{kc % 2}",
                                       name="psS")
                        for hi in range(2):
                            base = 64 * hi
                            nc.tensor.matmul(
                                psS[:, hi * NQ:(hi + 1) * NQ],
                                qk_sb[base:base + 64, FT + ft, kc * P:(kc + 1) * P],
                                qk_sb[base:base + 64, ft, qsl],
                                start=True, stop=True)
                        u_t = wk.tile([P, 2 * NQ], F16, tag="u", bufs=2)
                        nc.scalar.activation(u_t[:], psS[:], AF.Exp,
                                             bias=eshift[:])
                        u2 = wk.tile([P, 2 * NQ], F16, tag="u2", bufs=4)
                        nc.vector.tensor_mul(
                            u2[:].rearrange("p (a b) -> p a b", a=2),
                            u_t[:].rearrange("p (a b) -> p a b", a=2),
                            eb[:, kc:kc + 1, :].broadcast_to([P, 2, NQ]))
                        if u >= 13:
                            # dummies pace iters 0-7 (Act needs period >=
                            # ~1.15us); o_proj fills wait for norms at kc8
                            if kc < 8:
                                cum = kc + 1
                            else:
                                cum = 8 + ((n_mm - 8) * (kc - 7)) // 8
                        else:
                            cum = (n_mm * (kc + 1)) // KC
                        want = cum - done
                        done += want - pump(gens, want)
                        if kc in FIN_SLOTS and pending_fin:
                            pending_fin.pop(0)()
                        u2q.append(u2)
                        if kc >= 3:
                            if ctx is None:
                                ctx = pps.tile([P, 2 * NQ], F32, tag="ctx",
                                               name="ctx")
                            emit_pv(ctx, hp, kc - 3, u2q, stop=False)
                    for kp in (KC - 3, KC - 2, KC - 1):
                        emit_pv(ctx, hp, kp, u2q, stop=(kp == KC - 1))
                    while gens:  # leftover fill (shouldn't trigger)
                        pump(gens, 1 << 30)

                    # finalize, deferred into the next unit's odd-kc slots as
                    # small stages so no engine queue gets a multi-us clump:
                    # evict ctx (ScalarE), denom reciprocal via DRAM
                    # round-trip broadcast (HW DMA queues), normalize (Pool).
                    st = {}

                    def f_evict(ctx=ctx):
                        st["ctx_sb"] = wk.tile([65, 2 * NQ], F32, tag="ctxe",
                                               bufs=1, name="ctx_sb")
                        nc.vector.tensor_copy(st["ctx_sb"][:], ctx[0:65, :])

                    def f_rd():
                        st["rd"] = dpool.tile([2 * NQ], F32, name="rd")
                        nc.sync.dma_start(st["rd"][None, :],
                                          st["ctx_sb"][64:65, :])
                        st["rsq"] = wk.tile([32, 2 * NQ // 32], F32, tag="rsq",
                                            bufs=2, name="rsq")
                        nc.sync.dma_start(
                            st["rsq"][:], st["rd"].rearrange("(a b) -> a b",
                                                             a=32))

                    def f_recip():
                        st["rrec"] = wk.tile([32, 2 * NQ // 32], F16,
                                             tag="rrec", bufs=2, name="rrec")
                        with nc.allow_low_precision(reason="1/denom fp16"):
                            nc.vector.reciprocal(st["rrec"][:], st["rsq"][:])

                    def f_rb():
                        st["rd2"] = dpool.tile([2 * NQ], F16, name="rd2")
                        nc.sync.dma_start(
                            st["rd2"].rearrange("(a b) -> a b", a=32),
                            st["rrec"][:])
                        st["rb"] = wk.tile([64, 2 * NQ], F16, tag="rb", bufs=2, name="rb")
                        nc.sync.dma_start(st["rb"][:],
                                          st["rd2"].partition_broadcast(64))

                    def f_norm(hi, hp=hp, qsl=qsl):
                        base = 64 * hi
                        nc.gpsimd.tensor_mul(
                            ctxT[base:base + 64, hp, qsl],
                            st["ctx_sb"][0:64, hi * NQ:(hi + 1) * NQ],
                            st["rb"][:, hi * NQ:(hi + 1) * NQ])

                    pending_fin.extend([
                        f_evict, f_rd, f_recip, f_rb,
                        lambda: f_norm(0), lambda: f_norm(1)])

            # ---------- trail: finalize last unit + last o_proj column ----
            for f in pending_fin:
                f()
            pending_fin = []
            while TRAIL:
                pump(TRAIL, 1 << 30)

    nc.compile()
    return nc


def make_core_inputs(hidden_states, attention_bias, rope_cos, rope_sin,
                     head_mask, qkv_w, qkv_b, o_w, S=2048, D=1024):
    """Host-side sharding + layout prep. Returns list of 8 input dicts."""
    f32, f16 = np.float32, np.float16
    hidden_states = np.asarray(hidden_states, f32)
    attention_bias = np.asarray(attention_bias, f32)
    rope_cos = np.asarray(rope_cos, f32)
    rope_sin = np.asarray(rope_sin, f32)
    head_mask = np.asarray(head_mask, f32).reshape(-1)
    qkv_w = np.asarray(qkv_w, f32)
    qkv_b = np.asarray(qkv_b, f32)
    o_w = np.asarray(o_w, f32)

    FPC = HPC * 64
    F = H * 64

    # d-permutation: position p = 2*(d%32) + d//32  (rotate partners adjacent)
    perm = np.empty(64, np.int64)
    for d in range(64):
        perm[2 * (d % 32) + d // 32] = d
    x1 = np.arange(128) ^ 1      # partition pair-swap (within 64-halves too)
    sgn64 = np.where(np.arange(64) % 2 == 0, -1.0, 1.0).astype(f32)

    def perm_rows(w):
        # w: [FPC(, D)] rows f = h*64 + d -> rows h*64 + p with p-order
        w = w.reshape(HPC, 64, -1)
        out = w[:, perm]
        return out.reshape(HPC * 64, -1)

    cos64 = rope_cos[0, :, 0, :]           # [S, 64]
    sin64 = rope_sin[0, :, 0, :]
    cos_p = cos64[:, perm].T               # [64, S] p-order
    sin_p = (sin64[:, perm] * sgn64[None, :]).T
    cosr = np.concatenate([cos_p, cos_p], axis=0).astype(f16)   # [128, S]
    sinr = np.concatenate([sin_p, sin_p], axis=0).astype(f16)

    in_maps = []
    for c in range(8):
        b, g = divmod(c, G)
        fs = slice(g * FPC, (g + 1) * FPC)
        wq = perm_rows(qkv_w[F * 0:F * 1][fs])
        wk_ = perm_rows(qkv_w[F * 1:F * 2][fs])
        bq = perm_rows(qkv_b[F * 0:F * 1][fs, None]).ravel()
        bk = perm_rows(qkv_b[F * 1:F * 2][fs, None]).ravel()
        wv = qkv_w[F * 2:F * 3][fs].copy()
        bvv = qkv_b[F * 2:F * 3][fs].copy()
        mask = head_mask[g * HPC:(g + 1) * HPC]
        wv *= np.repeat(mask, 64)[:, None]
        bvv *= np.repeat(mask, 64)

        wqk = np.concatenate([wq.T, wk_.T], axis=1)   # [D, 2*FPC]
        # bias scalars [16 cols x 128]: (qk*4+ft)*2 + {plain, shuffled}
        bcols = np.empty((16, 128), f32)
        for qk, bvec in ((0, bq), (1, bk)):
            for ft in range(4):
                seg = bvec[ft * 128:(ft + 1) * 128]
                bcols[(qk * 4 + ft) * 2 + 0] = seg
                bcols[(qk * 4 + ft) * 2 + 1] = seg[x1]
        bT = np.ascontiguousarray(attention_bias[b, 0].T)
        m = {
            "hT": np.ascontiguousarray(hidden_states[b].T).astype(f16),
            "wqk": np.ascontiguousarray(wqk).astype(f16),
            "bqk": np.ascontiguousarray(bcols.ravel()),
            "wvT": np.ascontiguousarray(wv.T).astype(f16),
            "bv": np.ascontiguousarray(bvv).astype(f16),
            "cosr": np.ascontiguousarray(cosr),
            "sinr": np.ascontiguousarray(sinr),
            "expbT": np.exp(bT).astype(f16),
            "owT": np.ascontiguousarray(o_w[:, g * FPC:(g + 1) * FPC].T).astype(f16),
        }
        in_maps.append(m)
    return in_maps


def kernel(hidden_states, attention_bias, rope_cos, rope_sin, head_mask,
           qkv_w, qkv_b, o_w, o_b, **_unused):
    from concourse.bass_utils import run_bass_kernel_spmd

    B, S, D = hidden_states.shape
    if "nc" not in _CACHE:
        _CACHE["nc"] = build_nc(S=S, D=D)
    nc = _CACHE["nc"]

    in_maps = make_core_inputs(hidden_states, attention_bias, rope_cos,
                               rope_sin, head_mask, qkv_w, qkv_b, o_w,
                               S=S, D=D)
    res = run_bass_kernel_spmd(nc, in_maps, list(range(8)))
    _CACHE["last_results"] = res

    o_b = np.asarray(o_b, np.float32)
    out = np.empty((B, S, D), np.float32)
    for b in range(B):
        acc = res.results[2 * b]["outT"].T + res.results[2 * b + 1]["outT"].T
        out[b] = acc + o_b[None, :]
    return out
